# revision 22
# baseline (speedup 1.0000x reference)
"""ChebConv SpMM kernel for 8 TRN2 NeuronCores (gather + matmul segment-sum).

Strategy (dest-sharded graph-parallel):
- x held as fp16 [V, 128] (col = b*32 + fi). Core c owns dest rows
  [c*Vc, (c+1)*Vc), split in lo/hi halves of HALF rows.
- Each Chebyshev step, per 512-dest superwindow: gather x[cols] for its
  edges via dma_gather (fp16, 256B rows, 4 SWDGE queues round-robin),
  build one-hot*val selection tiles on DVE (is_equal vs a static iota),
  and matmul-accumulate contributions into PSUM per 128-dest window
  (PSUM pre-initialized with -x_{k-1} by DVE). No scatter-add at all.
- y written back sequentially (fp16); halves AllGathered chunk-wise into
  the next step's gather buffers (chunk-major layout) so collectives
  overlap compute.
- Final projection out = sum_k T_k W_k in fp16: PE transpose of cheb
  tiles + matmul against block-diagonal W, bias fused in the PSUM->SBUF
  copy.
- Edges are host-sorted into (dest-half, superwindow, source-region,
  window) groups, padded to 128 (program structure shared across cores;
  per-core trailing pads use idx=-1 so the gather ucode skips them).
"""
import sys

sys.path.insert(0, "/opt/trn_rl_repo")

import numpy as np

import concourse.bass as bass
import concourse.bacc as bacc
import concourse.mybir as mybir
import concourse.tile as tile
from concourse import bass_utils

F32 = mybir.dt.float32
F16 = mybir.dt.float16
I16 = mybir.dt.int16


# ----------------------------------------------------------------------------
# Host-side preprocessing
# ----------------------------------------------------------------------------

class Plan:
    pass


def build_plan(lap_rows, lap_cols, lap_vals, V, n_cores=8, region_rows=32768,
               SW=512, WIN=128, chunk_sws=(7, 7, 6, 3, 1), call_cap=1024):
    C = n_cores
    Vc = V // C
    HALF = Vc // 2
    BUFROWS = HALF * C
    NREG = (BUFROWS + region_rows - 1) // region_rows   # 3 regions per buffer
    NSW = HALF // SW                                     # superwindows per half
    WPS = SW // WIN                                      # windows per sw
    assert sum(chunk_sws) == NSW
    # AG chunk row boundaries within a half (uneven: small tail chunks so
    # the next step's gathers stall less on the last AllGather)
    cb = np.r_[0, np.cumsum(np.asarray(chunk_sws) * SW)]    # [NCH+1]
    rows_arr = np.arange(HALF)
    chunk_of = np.searchsorted(cb, rows_arr, side="right") - 1
    # xf row for (core cc, local row r): cb[ch]*C + cc*chlen + (r - cb[ch])
    chlen = (cb[chunk_of + 1] - cb[chunk_of])

    def chunkrow(cc, r):
        ch = chunk_of[r]
        return cb[ch] * C + cc * chlen[r] + (r - cb[ch])

    rows = np.asarray(lap_rows).astype(np.int64)
    cols = np.asarray(lap_cols).astype(np.int64)
    vals = np.asarray(lap_vals).astype(np.float32)

    # dest decomposition
    core = rows // Vc
    d = rows % Vc
    h = d // HALF
    dhh = d % HALF
    sw = dhh // SW
    wl = (dhh % SW) // WIN          # window within superwindow
    drel = dhh % WIN                # dest within window

    # source decomposition (chunk-major xf layout)
    cv = cols // Vc
    off = cols % Vc
    hs = off // HALF
    r = off % HALF
    srow = chunkrow(cv, r)
    region = hs * NREG + srow // region_rows             # 0..5
    gidx = srow % region_rows

    # group sizes: max over cores, rounded to 128
    cnt = np.zeros((C, 2, NSW, 2 * NREG, WPS), np.int64)
    np.add.at(cnt, (core, h, sw, region, wl), 1)
    gsz = cnt.max(axis=0)
    gsz = ((gsz + 127) // 128) * 128                     # [2, NSW, 6, WPS]

    # slot0 per group, iterated (h, sw, region, wl)
    flat = gsz.reshape(-1)
    slot0_flat = np.r_[0, np.cumsum(flat)[:-1]]
    slot0 = slot0_flat.reshape(gsz.shape)
    TOT = int(flat.sum())
    assert TOT % 128 == 0

    # per-edge slot assignment
    okey = (((h * NSW + sw) * (2 * NREG) + region) * WPS + wl) * C + core
    order = np.argsort(okey, kind="stable")
    ks = okey[order]
    starts = np.r_[0, np.nonzero(np.diff(ks))[0] + 1]
    gstart = np.repeat(starts, np.diff(np.r_[starts, len(ks)]))
    within = np.arange(len(ks)) - gstart
    slots = np.empty(len(ks), np.int64)
    slots = slot0[h[order], sw[order], region[order], wl[order]] + within

    g_arr = np.zeros((C, TOT), np.int16)
    v_arr = np.zeros((C, TOT), np.float16)
    d_arr = np.full((C, TOT), 300.0, np.float16)
    occ = np.zeros((C, TOT), bool)
    co = core[order]
    g_arr[co, slots] = gidx[order].astype(np.int16)
    v_arr[co, slots] = vals[order].astype(np.float16)
    d_arr[co, slots] = drel[order].astype(np.float16)
    occ[co, slots] = True

    # per-sw structure (identical across steps)
    plan = Plan()
    plan.sws = []
    ngr_max = 0
    for hh in range(2):
        for ss in range(NSW):
            s0 = int(slot0[hh, ss, 0, 0])
            n = int(gsz[hh, ss].sum())
            ngr = n // 128
            ngr_max = max(ngr, ngr_max)
            calls = []
            batch_wl = []
            c_local = 0
            for reg in range(2 * NREG):
                rn = int(gsz[hh, ss, reg].sum())
                npc = (rn + call_cap - 1) // call_cap    # balanced pieces
                p0 = 0
                for p in range(npc):
                    cn = (rn // npc // 128 * 128) + \
                        (128 if p < (rn // 128) % npc else 0)
                    calls.append((c_local + p0, cn, reg))
                    p0 += cn
                assert p0 == rn
                for w in range(WPS):
                    batch_wl += [w] * (int(gsz[hh, ss, reg, w]) // 128)
                c_local += rn
            assert len(batch_wl) == ngr
            plan.sws.append({"h": hh, "sw": ss, "slot0": s0, "ngr": ngr,
                             "calls": calls, "batch_wl": batch_wl})

    # trailing pads of each call -> idx -1 (gather ucode trims them)
    for swd in plan.sws:
        s0 = swd["slot0"]
        for (c0, cn, reg) in swd["calls"]:
            a, b = s0 + c0, s0 + c0 + cn
            seg = occ[:, a:b]
            has = seg.any(axis=1)
            last = np.where(has, seg.shape[1] - 1 - np.argmax(seg[:, ::-1],
                                                              axis=1), -1)
            mask = np.arange(b - a)[None, :] > last[:, None]
            blk = g_arr[:, a:b]
            blk[mask] = -1
            g_arr[:, a:b] = blk

    plan.TOT = TOT
    plan.C, plan.Vc, plan.HALF, plan.BUFROWS = C, Vc, HALF, BUFROWS
    plan.NREG, plan.REGION = NREG, region_rows
    plan.SW, plan.WIN, plan.WPS, plan.NSW = SW, WIN, WPS, NSW
    plan.chunk_sws = list(chunk_sws)
    plan.cb = cb                       # chunk row boundaries within a half
    plan.chunkrow = chunkrow
    plan.ngr_max = ngr_max
    plan.V = V

    def wrap16(a):  # [C, TOT] -> [C, 128, TOT//16] (16-wrap, replicated x8)
        w = a.reshape(C, TOT // 16, 16).transpose(0, 2, 1)
        return np.tile(w, (1, 8, 1)).copy()

    def wrap128(a):
        return a.reshape(C, TOT // 128, 128).transpose(0, 2, 1).copy()

    plan.gidx = wrap16(g_arr)
    plan.gvals = wrap128(v_arr)
    plan.gvals2 = wrap128((2.0 * v_arr.astype(np.float32)).astype(np.float16))
    plan.didx = wrap128(d_arr)
    return plan


def host_prep(lap_rows, lap_cols, lap_vals, inputs, weight, bias, n_cores=8,
              **plan_kw):
    B, V, FIN = inputs.shape
    K, _, FOUT = weight.shape
    CW = B * FIN
    plan = build_plan(lap_rows, lap_cols, lap_vals, V, n_cores, **plan_kw)
    plan.B, plan.FIN, plan.FOUT, plan.K, plan.CW = B, FIN, FOUT, K, CW
    x0 = np.ascontiguousarray(inputs.transpose(1, 0, 2).reshape(V, CW))
    x0h = x0.astype(np.float16)
    # chunk-major gather buffers for step 0 (uploaded, not AllGathered)
    C, HALF = n_cores, plan.HALF
    xf0 = np.empty((2, plan.BUFROWS, CW), np.float16)
    r = np.arange(HALF)
    for cc in range(C):
        srow = plan.chunkrow(cc, r)
        for h in range(2):
            xf0[h, srow] = x0h[cc * plan.Vc + h * HALF + r]
    # Reference pairs cheb col (fi, k) with weight.reshape(K*Fin, F)[fi*K + k]
    W_eff = np.asarray(weight, np.float32).reshape(K * FIN, FOUT) \
        .reshape(FIN, K, FOUT).transpose(1, 0, 2)
    Wblk = np.zeros((K, CW, B * FOUT), np.float16)
    for k in range(K):
        for b in range(B):
            Wblk[k, b * FIN:(b + 1) * FIN, b * FOUT:(b + 1) * FOUT] = \
                W_eff[k].astype(np.float16)
    bias_row = np.tile(np.tile(np.asarray(bias, np.float32), B)[None, :],
                       (128, 1))
    ident = np.eye(128, dtype=np.float16)
    iota = np.tile(np.arange(128, dtype=np.float16)[None, :], (128, 1))
    in_maps = []
    for c in range(n_cores):
        in_maps.append({
            "x0h": x0h[c * plan.Vc:(c + 1) * plan.Vc],
            "xf0_0": xf0[0], "xf0_1": xf0[1],
            "gidx": plan.gidx[c], "didx": plan.didx[c],
            "gvals": plan.gvals[c], "gvals2": plan.gvals2[c],
            "wblk": Wblk, "bias_row": bias_row, "ident": ident, "iota": iota,
        })
    return plan, in_maps


# ----------------------------------------------------------------------------
# Device program
# ----------------------------------------------------------------------------

def build_program(plan, n_cores=8):
    C, Vc, HALF, BUFROWS = plan.C, plan.Vc, plan.HALF, plan.BUFROWS
    REGION, NREG, TOT = plan.REGION, plan.NREG, plan.TOT
    K, B, FIN, FOUT, CW = plan.K, plan.B, plan.FIN, plan.FOUT, plan.CW
    SW, WIN, WPS, NSW = plan.SW, plan.WIN, plan.WPS, plan.NSW
    cb = plan.cb
    chunk_end_sw = list(np.cumsum(plan.chunk_sws))
    NGM = plan.ngr_max
    KS = K - 1
    BFO = B * FOUT

    nc = bacc.Bacc("TRN2", target_bir_lowering=False, debug=False,
                   num_devices=n_cores, num_swdge_queues=4)
    x0h = nc.dram_tensor("x0h", [Vc, CW], F16, kind="ExternalInput")
    xf0 = [nc.dram_tensor(f"xf0_{h}", [BUFROWS, CW], F16,
                          kind="ExternalInput") for h in range(2)]
    gidx = nc.dram_tensor("gidx", [128, TOT // 16], I16, kind="ExternalInput")
    didx = nc.dram_tensor("didx", [128, TOT // 128], F16, kind="ExternalInput")
    gvals = nc.dram_tensor("gvals", [128, TOT // 128], F16,
                           kind="ExternalInput")
    gvals2 = nc.dram_tensor("gvals2", [128, TOT // 128], F16,
                            kind="ExternalInput")
    wblk = nc.dram_tensor("wblk", [K, CW, BFO], F16, kind="ExternalInput")
    bias_row = nc.dram_tensor("bias_row", [128, BFO], F32,
                              kind="ExternalInput")
    ident = nc.dram_tensor("ident", [128, 128], F16, kind="ExternalInput")
    iota = nc.dram_tensor("iota", [128, 128], F16, kind="ExternalInput")
    out = nc.dram_tensor("out", [Vc, BFO], F32, kind="ExternalOutput")

    xf = [xf0] + [[nc.dram_tensor(f"xf{k}_{h}", [BUFROWS, CW], F16,
                                  addr_space="Shared")
                   for h in range(2)] for k in range(1, KS)]
    yb = {k: [nc.dram_tensor(f"y{k}_{h}", [HALF, CW], F16) for h in range(2)]
          for k in range(1, KS + 1)}

    qrr = [0]

    def rr():
        qrr[0] = (qrr[0] + 1) % 4
        return qrr[0]

    VSUP = min(1024, HALF)
    assert HALF % VSUP == 0 and Vc % VSUP == 0
    TSUB = VSUP // 128
    SW_PER_VSUP = VSUP // SW

    with tile.TileContext(nc) as tc:
        with tc.tile_pool(name="cst", bufs=1) as cst, \
             tc.tile_pool(name="gip", bufs=3) as gip, \
             tc.tile_pool(name="vdp", bufs=3) as vdp, \
             tc.tile_pool(name="ring", bufs=3) as ring, \
             tc.tile_pool(name="s2p", bufs=3) as s2p, \
             tc.tile_pool(name="xpp", bufs=3) as xpp, \
             tc.tile_pool(name="ytp", bufs=3) as ytp, \
             tc.tile_pool(name="pst", bufs=4, space="PSUM") as pst, \
             tc.tile_pool(name="cbp", bufs=2) as cbp, \
             tc.tile_pool(name="pst2", bufs=2, space="PSUM") as pst2, \
             tc.tile_pool(name="pso", bufs=2, space="PSUM") as pso, \
             tc.tile_pool(name="lhp", bufs=3) as lhp, \
             tc.tile_pool(name="stp", bufs=2) as stp:
            iot = cst.tile([128, 128], F16, tag="iota")
            nc.sync.dma_start(iot[:], iota[:])
            wt = []
            for k in range(K):
                wtk = cst.tile([CW, BFO], F16, tag=f"w{k}")
                wt.append(wtk)
                nc.sync.dma_start(wtk[:], wblk[k, :, :])
            bt = cst.tile([128, BFO], F32, tag="bias")
            nc.sync.dma_start(bt[:], bias_row[:])
            idt = cst.tile([128, 128], F16, tag="ident")
            nc.sync.dma_start(idt[:], ident[:])
            # pre-zero ring bufs (pad slots are never gathered; stale NaNs
            # would poison matmuls through S=0 otherwise)
            for _ in range(3):
                t = ring.tile([128, NGM, CW], F16, tag="rg")
                nc.vector.memset(t[:], 0.0)

            def project(sc):
                """Emit projection for 1024-row super-chunk sc."""
                vbase = sc * VSUP
                h, hb = (0, vbase) if vbase < HALF else (1, vbase - HALF)
                cb = []
                for k in range(K):
                    cbt = cbp.tile([128, TSUB, CW], F16, tag=f"cb{k}")
                    srcv = x0h[vbase:vbase + VSUP, :] if k == 0 else \
                        yb[k][h][hb:hb + VSUP, :]
                    nc.sync.dma_start(
                        cbt[:], srcv.rearrange("(ts p) c -> p ts c", p=128))
                    cb.append(cbt)
                st = stp.tile([128, TSUB, BFO], F32, tag="stage")
                for ts in range(TSUB):
                    po = pso.tile([128, BFO], F32, tag="po")
                    for k in range(K):
                        ptt = pst2.tile([128, 128], F16, tag="pt")
                        nc.tensor.transpose(ptt[:, :CW], cb[k][:, ts, :],
                                            idt[:])
                        lh = lhp.tile([128, CW], F16, tag="lh")
                        nc.vector.tensor_copy(lh[:, :CW], ptt[:, :CW])
                        nc.tensor.matmul(po[:], lh[:, :CW], wt[k][:],
                                         start=(k == 0), stop=(k == K - 1))
                    nc.vector.tensor_tensor(
                        st[:, ts, :], po[:], bt[:], mybir.AluOpType.add)
                nc.sync.dma_start(
                    out[vbase:vbase + VSUP, :].rearrange("(ts p) c -> p ts c",
                                                         p=128), st[:])

            for k in range(KS):               # step k computes x_{k+1}
                vsrc = gvals if k == 0 else gvals2
                for swd in plan.sws:
                    h, ss = swd["h"], swd["sw"]
                    s0, ngr = swd["slot0"], swd["ngr"]
                    n = ngr * 128
                    gi = gip.tile([128, NGM * 8], I16, tag="gi")
                    nc.sync.dma_start(gi[:, :n // 16],
                                      gidx[:, s0 // 16:(s0 + n) // 16])
                    dv = vdp.tile([128, NGM], F16, tag="dv")
                    nc.sync.dma_start(dv[:, :ngr],
                                      didx[:, s0 // 128:(s0 + n) // 128])
                    vt = vdp.tile([128, NGM], F16, tag="vt")
                    nc.sync.dma_start(vt[:, :ngr],
                                      vsrc[:, s0 // 128:(s0 + n) // 128])
                    rg = ring.tile([128, NGM, CW], F16, tag="rg")
                    for (c0, cn, reg) in swd["calls"]:
                        srcap = xf[k][reg // NREG][
                            (reg % NREG) * REGION:
                            min((reg % NREG + 1) * REGION, BUFROWS), :]
                        nc.gpsimd.dma_gather(
                            rg[:, c0 // 128:(c0 + cn) // 128, :], srcap,
                            gi[:, c0 // 16:(c0 + cn) // 16], cn, cn, CW,
                            queue_num=rr())
                    s2 = s2p.tile([128, NGM, WIN], F16, tag="s2")
                    nc.vector.tensor_tensor(
                        s2[:, :ngr, :],
                        dv[:, :ngr].unsqueeze(-1).broadcast_to(
                            [128, ngr, WIN]),
                        iot[:].unsqueeze(1).broadcast_to([128, ngr, WIN]),
                        mybir.AluOpType.is_equal)
                    nc.vector.tensor_tensor(
                        s2[:, :ngr, :], s2[:, :ngr, :],
                        vt[:, :ngr].unsqueeze(-1).broadcast_to(
                            [128, ngr, WIN]),
                        mybir.AluOpType.mult)
                    pt = pst.tile([128, WPS, WIN], F32, tag="pt")
                    if k == 0:
                        nc.vector.memset(pt[:], 0.0)
                    else:
                        xp = xpp.tile([128, WPS, WIN], F16, tag="xp")
                        if k == 1:
                            srcv = x0h[h * HALF + ss * SW:
                                       h * HALF + (ss + 1) * SW, :]
                        else:
                            srcv = yb[k - 1][h][ss * SW:(ss + 1) * SW, :]
                        nc.sync.dma_start(
                            xp[:], srcv.rearrange("(g p) c -> p g c", p=128))
                        nc.scalar.mul(pt[:], xp[:], -1.0)
                    for b, w in enumerate(swd["batch_wl"]):
                        nc.tensor.matmul(pt[:, w, :], s2[:, b, :],
                                         rg[:, b, :], start=False, stop=False,
                                         skip_group_check=True)
                    yt = ytp.tile([128, WPS, WIN], F16, tag="yt")
                    nc.scalar.copy(yt[:], pt[:])
                    nc.sync.dma_start(
                        yb[k + 1][h][ss * SW:(ss + 1) * SW, :].rearrange(
                            "(g p) c -> p g c", p=128), yt[:])
                    if k < KS - 1 and (ss + 1) in chunk_end_sw:
                        j = chunk_end_sw.index(ss + 1)
                        r0, r1 = int(cb[j]), int(cb[j + 1])
                        nc.gpsimd.collective_compute(
                            "AllGather", mybir.AluOpType.bypass,
                            replica_groups=[list(range(n_cores))],
                            ins=[yb[k + 1][h][r0:r1, :].opt()],
                            outs=[xf[k + 1][h][r0 * C:r1 * C, :].opt()])
                    # last step: interleave the final projection per VSUP rows
                    if k == KS - 1 and (ss + 1) % SW_PER_VSUP == 0:
                        project(h * (HALF // VSUP) +
                                (ss + 1) // SW_PER_VSUP - 1)
    nc.compile()
    return nc


# ----------------------------------------------------------------------------
# Self-contained entry point (full inputs in, full output out)
# ----------------------------------------------------------------------------

_CACHE = {}


def kernel(lap_rows, lap_cols, lap_vals, inputs, weight, bias):
    """ChebConv on 8 TRN2 NeuronCores. Full inputs -> full [B, V, FOUT]."""
    inputs = np.asarray(inputs, np.float32)
    weight = np.asarray(weight, np.float32)
    bias = np.asarray(bias, np.float32)
    lap_rows = np.asarray(lap_rows)
    lap_cols = np.asarray(lap_cols)
    lap_vals = np.asarray(lap_vals, np.float32)
    B, V, FIN = inputs.shape
    K, _, FOUT = weight.shape
    n_cores = 8
    plan, in_maps = host_prep(lap_rows, lap_cols, lap_vals, inputs, weight,
                              bias, n_cores)
    key = ("v2", V, plan.TOT, plan.ngr_max,
           tuple(swd["ngr"] for swd in plan.sws))
    if key not in _CACHE:
        _CACHE.clear()
        _CACHE[key] = build_program(plan, n_cores)
    nc = _CACHE[key]
    res = bass_utils.run_bass_kernel_spmd(nc, in_maps,
                                          core_ids=list(range(n_cores)))
    outs = []
    for c in range(n_cores):
        o = res.results[c]["out"]
        outs.append(o.reshape(plan.Vc, B, FOUT).transpose(1, 0, 2))
    return np.ascontiguousarray(np.concatenate(outs, axis=1)).astype(np.float32)
